# revision 10
# baseline (speedup 1.0000x reference)
"""LSTM decoder (constant input per step, ragged lengths) on 8 TRN2 cores.

Math (per batch element b, for t < seq_len[b]):
    x_proj = Z @ W_ih.T + b_ih + b_hh            (constant over time)
    gates_t = x_proj + h_t @ W_hh.T
    i,f,g,o = split(gates_t); c = sig(f)*c + sig(i)*tanh(g); h = sig(o)*tanh(c)
    ys[b, t] = h_{t+1}

The recurrence is chaotic: bf16 (or tf32-class) state/weights diverge from the
fp32 reference by O(1) after ~500 steps, so everything runs in fp32.

Device strategy (data-parallel over batch, 16 sequences per core):
  * Streaming matmul form: stationary = h.T chunks [128, 16] (cheap reloads),
    moving = W_hh.T column blocks [128, 512] at fp32 (4 cycles/row).
    Gates come out in "layout 1": [batch(16) partitions, gate free].
  * W columns are reordered host-side so N-chunk n holds the i|f|g|o gates of
    hidden block n (128 units) - each chunk's elementwise finishes early and
    its h block is transposed (PE transpose) into the next step's stationary.
  * x_proj computed once on device (fp32), bias folded in host-side layout.
"""

import numpy as np

import concourse.bass as bass
import concourse.tile as tile
from concourse import bacc, mybir
from concourse.bass_utils import run_bass_kernel_spmd

B, F, H, TMAX = 128, 128, 512, 512
N_CORES = 8
BL = B // N_CORES          # local batch = 16
NB = 4                     # hidden blocks of 128 (= N chunks and K chunks)
T_STEPS = TMAX - 1         # seq_len < 512, so at most 511 steps matter

FP32 = mybir.dt.float32
AF = mybir.ActivationFunctionType


def build_lstm_nc(t_steps: int = T_STEPS):
    """Build + compile the per-core Bass program (SPMD: same NEFF, 8 cores)."""
    nc = bacc.Bacc("TRN2", target_bir_lowering=False, debug=False)

    # W_hh.T with reordered columns: wr[:, k*2048 + n*512 + g*128 + q]
    #   = W_hh[g*512 + 128n + q, 128k + p]
    wr_d = nc.dram_tensor("wr", [128, NB * 2048], FP32, kind="ExternalInput")
    # W_ih.T with the same column reorder (single K chunk, F=128)
    wi_d = nc.dram_tensor("wi", [128, 2048], FP32, kind="ExternalInput")
    z_d = nc.dram_tensor("z", [128, BL], FP32, kind="ExternalInput")
    bias_d = nc.dram_tensor("bias", [BL, 2048], FP32, kind="ExternalInput")
    eye_d = nc.dram_tensor("eye", [BL, BL], FP32, kind="ExternalInput")
    ys_d = nc.dram_tensor("ys", [t_steps, BL, H], FP32, kind="ExternalOutput")

    with tile.TileContext(nc) as tc:
        with (
            tc.tile_pool(name="const", bufs=1) as constp,
            tc.tile_pool(name="state", bufs=1) as statep,
            tc.tile_pool(name="work", bufs=3) as workp,
            tc.tile_pool(name="hout", bufs=4) as houtp,
            tc.tile_pool(name="ps", bufs=3, space="PSUM") as psp,
            tc.tile_pool(name="pst", bufs=2, space="PSUM") as pstp,
        ):
            # --- constants ---
            wr = constp.tile([128, NB * 2048], FP32)
            nc.sync.dma_start(wr[:], wr_d.ap())
            wi = constp.tile([128, 2048], FP32)
            nc.sync.dma_start(wi[:], wi_d.ap())
            z2 = constp.tile([128, BL], FP32)
            nc.sync.dma_start(z2[:], z_d.ap())
            bias = constp.tile([BL, 2048], FP32)
            nc.sync.dma_start(bias[:BL, :], bias_d.ap())
            eye = constp.tile([BL, BL], FP32)
            nc.sync.dma_start(eye[:BL, :], eye_d.ap())

            # --- x_proj (fp32, once): xp1[b, col] = (Z @ W_ih.T)[b, colmap] + bias ---
            xp1 = constp.tile([BL, 2048], FP32)
            for n in range(NB):
                xps = psp.tile([BL, 512], FP32, tag="xps")
                nc.tensor.matmul(
                    xps[:BL, :], z2[:], wi[:, n * 512 : (n + 1) * 512],
                    start=True, stop=True,
                )
                nc.vector.tensor_add(
                    xp1[:BL, n * 512 : (n + 1) * 512],
                    xps[:BL, :],
                    bias[:BL, n * 512 : (n + 1) * 512],
                )

            # --- state ---
            c1 = statep.tile([BL, H], FP32)          # cell, layout 1
            nc.vector.memset(c1[:BL, :], 0.0)
            hT = [
                statep.tile([128, NB * BL], FP32, tag=f"hT{j}", name=f"hT{j}")
                for j in range(2)
            ]  # h.T chunks: cols [16k:16k+16] = chunk k
            nc.vector.memset(hT[0][:], 0.0)

            # --- recurrence ---
            for t in range(t_steps):
                h_prev = hT[t % 2]
                h_next = hT[(t + 1) % 2]
                h1 = houtp.tile([BL, H], FP32, tag="h1")
                for n in range(NB):
                    ps = psp.tile([BL, 512], FP32, tag="gates")
                    for k in range(NB):
                        nc.tensor.matmul(
                            ps[:BL, :],
                            h_prev[:, k * BL : (k + 1) * BL],
                            wr[:, k * 2048 + n * 512 : k * 2048 + (n + 1) * 512],
                            start=(k == 0),
                            stop=(k == NB - 1),
                        )
                    # elementwise for hidden block n: chunk = [i|f|g|o] x 128
                    ga = workp.tile([BL, 512], FP32, tag="ga")
                    nc.vector.tensor_add(
                        ga[:BL, :], ps[:BL, :], xp1[:BL, n * 512 : (n + 1) * 512]
                    )
                    act = workp.tile([BL, 512], FP32, tag="act")
                    nc.scalar.activation(act[:BL, 0:256], ga[:BL, 0:256], AF.Sigmoid)
                    nc.scalar.activation(act[:BL, 256:384], ga[:BL, 256:384], AF.Tanh)
                    nc.scalar.activation(act[:BL, 384:512], ga[:BL, 384:512], AF.Sigmoid)
                    i_s = act[:BL, 0:128]
                    f_s = act[:BL, 128:256]
                    g_s = act[:BL, 256:384]
                    o_s = act[:BL, 384:512]
                    cn = c1[:BL, n * 128 : (n + 1) * 128]
                    t1 = workp.tile([BL, 128], FP32, tag="t1")
                    nc.vector.tensor_mul(t1[:BL, :], i_s, g_s)
                    nc.vector.tensor_mul(cn, f_s, cn)
                    nc.vector.tensor_add(cn, cn, t1[:BL, :])
                    tct = workp.tile([BL, 128], FP32, tag="tct")
                    nc.scalar.activation(tct[:BL, :], cn, AF.Tanh)
                    hn = h1[:BL, n * 128 : (n + 1) * 128]
                    nc.vector.tensor_mul(hn, o_s, tct[:BL, :])
                    # transpose h block n -> next step's stationary chunk n
                    psT = pstp.tile([128, BL], FP32, tag="psT")
                    nc.tensor.transpose(psT[:, :], hn, eye[:BL, :])
                    nc.vector.tensor_copy(h_next[:, n * BL : (n + 1) * BL], psT[:, :])
                    nc.sync.dma_start(
                        ys_d.ap()[t, :, n * 128 : (n + 1) * 128], hn
                    )

    nc.compile()
    return nc


def _prep_host_inputs(Z, seq_len, W_ih, W_hh, b_ih, b_hh):
    """Per-core in_maps with device-native layouts (fp32 end to end)."""
    WT = np.ascontiguousarray(W_hh.astype(np.float32).T)      # [H, 4H] (hid_in, gate)
    WIT = np.ascontiguousarray(W_ih.astype(np.float32).T)     # [F, 4H]
    bias = (b_ih.astype(np.float32) + b_hh.astype(np.float32))

    # column reorder: col = n*512 + g*128 + q  <->  gate index g*512 + 128n + q
    n_i = np.arange(2048)
    nn, rem = np.divmod(n_i, 512)
    gg, qq = np.divmod(rem, 128)
    colmap = gg * H + 128 * nn + qq                           # [2048]

    wr_np = np.empty((128, NB * 2048), dtype=np.float32)
    for k in range(NB):
        wr_np[:, k * 2048 : (k + 1) * 2048] = WT[k * 128 : (k + 1) * 128, colmap]
    wi_np = np.ascontiguousarray(WIT[:, colmap])
    bias_np = np.broadcast_to(bias[colmap], (BL, 2048)).copy()
    eye_np = np.eye(BL, dtype=np.float32)

    in_maps = []
    for c in range(N_CORES):
        zc = np.ascontiguousarray(Z[c * BL : (c + 1) * BL].astype(np.float32).T)
        in_maps.append(
            {"wr": wr_np, "wi": wi_np, "z": zc, "bias": bias_np, "eye": eye_np}
        )
    return in_maps


_NC_CACHE = {}


def get_nc(t_steps: int = T_STEPS):
    if t_steps not in _NC_CACHE:
        _NC_CACHE[t_steps] = build_lstm_nc(t_steps)
    return _NC_CACHE[t_steps]


def kernel(Z, seq_len, W_ih, W_hh, b_ih, b_hh, _trace=False, _tmpdir=None):
    nc = get_nc()
    in_maps = _prep_host_inputs(Z, seq_len, W_ih, W_hh, b_ih, b_hh)
    res = run_bass_kernel_spmd(
        nc, in_maps, core_ids=list(range(N_CORES)), trace=_trace, tmpdir=_tmpdir
    )
    kernel.last_result = res

    out = np.zeros((B, TMAX, H), dtype=np.float32)
    for c in range(N_CORES):
        ys = res.results[c]["ys"]  # [T_STEPS, BL, H] — already batch-major, hid order
        out[c * BL : (c + 1) * BL, :T_STEPS] = ys.transpose(1, 0, 2)
    mask = np.arange(TMAX, dtype=np.int64)[None, :] < seq_len.astype(np.int64)[:, None]
    out *= mask[:, :, None].astype(np.float32)
    return out


# revision 13
# speedup vs baseline: 1.2117x; 1.2117x over previous
"""LSTM decoder (constant input per step, ragged lengths) on 8 TRN2 cores.

Math (per batch element b, for t < seq_len[b]):
    x_proj = Z @ W_ih.T + b_ih + b_hh            (constant over time)
    gates_t = x_proj + h_t @ W_hh.T
    i,f,g,o = split(gates_t); c = sig(f)*c + sig(i)*tanh(g); h = sig(o)*tanh(c)
    ys[b, t] = h_{t+1}

The recurrence is chaotic: bf16/tf32-class rounding of h or W diverges from the
fp32 reference by O(1) after ~500 steps, so products must be fp32-exact.

Device strategy (data-parallel over batch, 16 sequences per core):
  * Streaming matmul form: stationary = h.T chunks [128, 16], moving = W_hh.T
    column blocks [128, 512].  Native fp32 matmul costs 4 cycles/row; instead
    both operands are Veltkamp-split into two ~12-bit-mantissa pieces which
    float32r (1 cycle/row at N>=512) multiplies EXACTLY, and the product is
    reconstructed in 3 accumulating passes (hi*hi + hi*lo + lo*hi, fp32 PSUM):
    verified 1.3e-7 relative vs fp64 on hardware, 25% less PE time than fp32.
  * Gates come out in "layout 1": [batch(16) partitions, gate free].  W columns
    are reordered host-side so N-chunk n holds the i|f|g|o gates of hidden
    block n (128 units): each chunk's elementwise finishes early, its h block
    is PE-transposed and split for the next step's stationary.
  * x_proj computed once on device the same way; bias added from a host tile.
"""

import numpy as np

import concourse.bass as bass
import concourse.tile as tile
from concourse import bacc, mybir
from concourse.bass_utils import run_bass_kernel_spmd

B, F, H, TMAX = 128, 128, 512, 512
N_CORES = 8
BL = B // N_CORES          # local batch = 16
NB = 4                     # hidden blocks of 128 (= N chunks and K chunks)
T_STEPS = TMAX - 1         # seq_len < 512, so at most 511 steps matter
SPLIT_C = float(2.0 ** 12 + 1)

FP32 = mybir.dt.float32
FP32R = mybir.dt.float32r
AF = mybir.ActivationFunctionType


def _split12(x):
    x = x.astype(np.float32)
    v = (x * np.float32(SPLIT_C)).astype(np.float32)
    hi = (v - (v - x).astype(np.float32)).astype(np.float32)
    lo = (x - hi).astype(np.float32)
    return hi, lo


def build_lstm_nc(t_steps: int = T_STEPS):
    """Build + compile the per-core Bass program (SPMD: same NEFF, 8 cores)."""
    nc = bacc.Bacc("TRN2", target_bir_lowering=False, debug=False)

    # W_hh.T, columns reordered and hi/lo split:
    #   wr*[:, k*2048 + n*512 + g*128 + q] = split(W_hh[g*512+128n+q, 128k+p])
    wrh_d = nc.dram_tensor("wrh", [128, NB * 2048], FP32R, kind="ExternalInput")
    wrl_d = nc.dram_tensor("wrl", [128, NB * 2048], FP32R, kind="ExternalInput")
    # W_ih.T with the same column reorder, hi/lo (single K chunk, F=128)
    wih_d = nc.dram_tensor("wih", [128, 2048], FP32R, kind="ExternalInput")
    wil_d = nc.dram_tensor("wil", [128, 2048], FP32R, kind="ExternalInput")
    z_d = nc.dram_tensor("z", [128, 2 * BL], FP32R, kind="ExternalInput")  # [hi|lo]
    bias_d = nc.dram_tensor("bias", [BL, 2048], FP32, kind="ExternalInput")
    eye_d = nc.dram_tensor("eye", [BL, BL], FP32, kind="ExternalInput")
    ys_d = nc.dram_tensor("ys", [t_steps, BL, H], FP32, kind="ExternalOutput")

    with tile.TileContext(nc) as tc:
        with (
            tc.tile_pool(name="const", bufs=1) as constp,
            tc.tile_pool(name="state", bufs=1) as statep,
            tc.tile_pool(name="work", bufs=3) as workp,
            tc.tile_pool(name="hout", bufs=4) as houtp,
            tc.tile_pool(name="ps", bufs=3, space="PSUM") as psp,
            tc.tile_pool(name="pst", bufs=2, space="PSUM") as pstp,
        ):
            # --- constants ---
            wrh = constp.tile([128, NB * 2048], FP32R)
            nc.sync.dma_start(wrh[:], wrh_d.ap())
            wrl = constp.tile([128, NB * 2048], FP32R)
            nc.sync.dma_start(wrl[:], wrl_d.ap())
            wih = constp.tile([128, 2048], FP32R)
            nc.sync.dma_start(wih[:], wih_d.ap())
            wil = constp.tile([128, 2048], FP32R)
            nc.sync.dma_start(wil[:], wil_d.ap())
            z2 = constp.tile([128, 2 * BL], FP32R)
            nc.sync.dma_start(z2[:], z_d.ap())
            bias = constp.tile([BL, 2048], FP32)
            nc.sync.dma_start(bias[:BL, :], bias_d.ap())
            eye = constp.tile([BL, BL], FP32)
            nc.sync.dma_start(eye[:BL, :], eye_d.ap())

            # --- x_proj (once): 3-pass exact product + bias ---
            xp1 = constp.tile([BL, 2048], FP32)
            z_hi = z2[:, :BL]
            z_lo = z2[:, BL:]
            for n in range(NB):
                xps = psp.tile([BL, 512], FP32, tag="xps")
                wi_h = wih[:, n * 512 : (n + 1) * 512]
                wi_l = wil[:, n * 512 : (n + 1) * 512]
                nc.tensor.matmul(xps[:BL, :], z_hi, wi_h, start=True, stop=False)
                nc.tensor.matmul(xps[:BL, :], z_hi, wi_l, start=False, stop=False)
                nc.tensor.matmul(xps[:BL, :], z_lo, wi_h, start=False, stop=True)
                nc.vector.tensor_add(
                    xp1[:BL, n * 512 : (n + 1) * 512],
                    xps[:BL, :],
                    bias[:BL, n * 512 : (n + 1) * 512],
                )

            # --- state ---
            c1 = statep.tile([BL, H], FP32)          # cell, layout 1
            nc.vector.memset(c1[:BL, :], 0.0)
            hTh = [
                statep.tile([128, NB * BL], FP32R, tag=f"hTh{j}", name=f"hTh{j}")
                for j in range(2)
            ]
            hTl = [
                statep.tile([128, NB * BL], FP32R, tag=f"hTl{j}", name=f"hTl{j}")
                for j in range(2)
            ]
            zf = statep.tile([128, NB * BL], FP32)
            nc.vector.memset(zf[:], 0.0)
            nc.vector.tensor_copy(hTh[0][:], zf[:])
            nc.vector.tensor_copy(hTl[0][:], zf[:])

            # --- recurrence ---
            for t in range(t_steps):
                hh_p, hl_p = hTh[t % 2], hTl[t % 2]
                hh_n, hl_n = hTh[(t + 1) % 2], hTl[(t + 1) % 2]
                h1 = houtp.tile([BL, H], FP32, tag="h1")
                for n in range(NB):
                    ps = psp.tile([BL, 512], FP32, tag="gates")
                    for k in range(NB):
                        w_h = wrh[:, k * 2048 + n * 512 : k * 2048 + (n + 1) * 512]
                        w_l = wrl[:, k * 2048 + n * 512 : k * 2048 + (n + 1) * 512]
                        s_h = hh_p[:, k * BL : (k + 1) * BL]
                        s_l = hl_p[:, k * BL : (k + 1) * BL]
                        nc.tensor.matmul(ps[:BL, :], s_h, w_h,
                                         start=(k == 0), stop=False)
                        nc.tensor.matmul(ps[:BL, :], s_h, w_l, start=False, stop=False)
                        nc.tensor.matmul(ps[:BL, :], s_l, w_h,
                                         start=False, stop=(k == NB - 1))
                    # elementwise for hidden block n: chunk = [i|f|g|o] x 128
                    ga = workp.tile([BL, 512], FP32, tag="ga")
                    nc.vector.tensor_add(
                        ga[:BL, :], ps[:BL, :], xp1[:BL, n * 512 : (n + 1) * 512]
                    )
                    act = workp.tile([BL, 512], FP32, tag="act")
                    nc.scalar.activation(act[:BL, 0:256], ga[:BL, 0:256], AF.Sigmoid)
                    nc.scalar.activation(act[:BL, 256:384], ga[:BL, 256:384], AF.Tanh)
                    nc.scalar.activation(act[:BL, 384:512], ga[:BL, 384:512], AF.Sigmoid)
                    i_s = act[:BL, 0:128]
                    f_s = act[:BL, 128:256]
                    g_s = act[:BL, 256:384]
                    o_s = act[:BL, 384:512]
                    cn = c1[:BL, n * 128 : (n + 1) * 128]
                    t1 = workp.tile([BL, 128], FP32, tag="t1")
                    nc.vector.tensor_mul(t1[:BL, :], i_s, g_s)
                    nc.vector.tensor_mul(cn, f_s, cn)
                    nc.vector.tensor_add(cn, cn, t1[:BL, :])
                    tct = workp.tile([BL, 128], FP32, tag="tct")
                    nc.scalar.activation(tct[:BL, :], cn, AF.Tanh)
                    hn = h1[:BL, n * 128 : (n + 1) * 128]
                    nc.vector.tensor_mul(hn, o_s, tct[:BL, :])
                    # transpose h block n, then split: hi = fp32r(h), lo = h - hi
                    psT = pstp.tile([128, BL], FP32, tag="psT")
                    nc.tensor.transpose(psT[:, :], hn, eye[:BL, :])
                    hi_n = hh_n[:, n * BL : (n + 1) * BL]
                    nc.vector.tensor_copy(hi_n, psT[:, :])
                    nc.vector.tensor_sub(hl_n[:, n * BL : (n + 1) * BL], psT[:, :], hi_n)
                    nc.sync.dma_start(
                        ys_d.ap()[t, :, n * 128 : (n + 1) * 128], hn
                    )

    nc.compile()
    return nc


def _prep_host_inputs(Z, seq_len, W_ih, W_hh, b_ih, b_hh):
    """Per-core in_maps with device-native layouts."""
    WT = np.ascontiguousarray(W_hh.astype(np.float32).T)      # [H, 4H] (hid_in, gate)
    WIT = np.ascontiguousarray(W_ih.astype(np.float32).T)     # [F, 4H]
    bias = (b_ih.astype(np.float32) + b_hh.astype(np.float32))

    # column reorder: col = n*512 + g*128 + q  <->  gate index g*512 + 128n + q
    n_i = np.arange(2048)
    nn, rem = np.divmod(n_i, 512)
    gg, qq = np.divmod(rem, 128)
    colmap = gg * H + 128 * nn + qq                           # [2048]

    wr_np = np.empty((128, NB * 2048), dtype=np.float32)
    for k in range(NB):
        wr_np[:, k * 2048 : (k + 1) * 2048] = WT[k * 128 : (k + 1) * 128, colmap]
    wrh_np, wrl_np = _split12(wr_np)
    wih_np, wil_np = _split12(np.ascontiguousarray(WIT[:, colmap]))
    bias_np = np.broadcast_to(bias[colmap], (BL, 2048)).copy()
    eye_np = np.eye(BL, dtype=np.float32)

    in_maps = []
    for c in range(N_CORES):
        zc = np.ascontiguousarray(Z[c * BL : (c + 1) * BL].astype(np.float32).T)
        z_hi, z_lo = _split12(zc)
        z_np = np.concatenate([z_hi, z_lo], axis=1)
        in_maps.append(
            {"wrh": wrh_np, "wrl": wrl_np, "wih": wih_np, "wil": wil_np,
             "z": z_np, "bias": bias_np, "eye": eye_np}
        )
    return in_maps


_NC_CACHE = {}


def get_nc(t_steps: int = T_STEPS):
    if t_steps not in _NC_CACHE:
        _NC_CACHE[t_steps] = build_lstm_nc(t_steps)
    return _NC_CACHE[t_steps]


def kernel(Z, seq_len, W_ih, W_hh, b_ih, b_hh, _trace=False, _tmpdir=None):
    nc = get_nc()
    in_maps = _prep_host_inputs(Z, seq_len, W_ih, W_hh, b_ih, b_hh)
    res = run_bass_kernel_spmd(
        nc, in_maps, core_ids=list(range(N_CORES)), trace=_trace, tmpdir=_tmpdir
    )
    kernel.last_result = res

    out = np.zeros((B, TMAX, H), dtype=np.float32)
    for c in range(N_CORES):
        ys = res.results[c]["ys"]  # [T_STEPS, BL, H] — batch-major, natural hid order
        out[c * BL : (c + 1) * BL, :T_STEPS] = ys.transpose(1, 0, 2)
    mask = np.arange(TMAX, dtype=np.int64)[None, :] < seq_len.astype(np.int64)[:, None]
    out *= mask[:, :, None].astype(np.float32)
    return out
